# revision 4
# baseline (speedup 1.0000x reference)
"""Bresenham (border-ring) attention kernel for Trainium2, 8 NeuronCores.

Computation (per full input):
    att  = einsum('bchw,c->bhw', x, w) + b        # 1x1 conv to 1 channel
    att  = sigmoid(att)
    mask = border ring of the HxW rectangle       # 1 on border, 0 inside
    out  = x * (att * (1 + mask))[:, None]

The op moves ~2 bytes of HBM traffic per FLOP — a pure bandwidth problem.
Whole pipeline in fp16 (x, weights, attention, output): halves HBM traffic
to ~103 MB/core (~287 us floor at the ~358 GB/s per-NC HBM limit); rel err
(max-abs / absmax) ~1e-3 vs the 2e-2 gate.

Engine notes discovered on this part:
  - PE array runs at the 1.2 GHz p-state (never 2.4 GHz boost); fp16
    N=512 matmuls: ~330 ns at M=1, ~533 ns at M=128.
  - DVE tensor_tensor and ANY GpSimd op serialize on the shared SBUF
    port pair (exclusive full-instruction lock), so the gpsimd
    partition_broadcast variant lost ~80 us to that lock — broadcast
    is done on the PE instead (it has slack at 3 matmuls/subtile).
  - TRN2 matmuls can only write f32 PSUM, so a PSUM->SBUF fp16 cast is
    needed before the DVE multiplies (fp16 keeps them in 2x mode); the
    casts are split ACT/DVE so neither engine exceeds the DMA window.

Per 512-col subtile (x7 per FD=3584 superblock, x2 batch, x14 blocks):
    PE : att = w0.x0 + w1.x1    2 matmuls [128,1]^T@[128,512] -> [1,512]
    ACT: st = sigmoid(att + b)  PSUM -> SBUF fp16
    DVE: s2 = st * m2           (m2 = 1+mask in {1,2}, [1,512] fp16 2x)
    PE : bcast ones[1,128]^T @ s2 -> psB [128,512] f32
    ACT (j even) / DVE (j odd): cmb = fp16(psB)
    DVE: ot[h] = xt[h] * cmb    2x [128,512] fp16 2x-mode multiplies
Stores split per half-block so ot frees early; loads full 1.79 MB.

Engine budget per core (28 blk-iters, ~10.3 us DMA each): DMA ~287 us
(bound), PE ~234 us, DVE ~224 us, ACT ~199 us, GpSimd ~0.
"""

import numpy as np

import concourse.bacc as bacc
import concourse.bass as bass
import concourse.tile as tile
from concourse import mybir
from concourse.bass_utils import run_bass_kernel_spmd

B, C, H, W = 16, 256, 224, 224
HW = H * W  # 50176
NCORES = 8
BLOC = B // NCORES  # 2

FD = 3584            # superblock free dim (spatial columns per tile)
SUB = 512            # matmul subtile (one PSUM bank of f32)
NSUB = FD // SUB     # 7
NBLK = HW // FD      # 14
HALF = FD // 2       # 1792 (store granularity)

F16 = mybir.dt.float16
F32 = mybir.dt.float32

# stash of the last BassKernelResults (test.py reads exec_time_ns from here)
LAST_RESULTS = None
_NC_CACHE = {}


def _build_nc():
    nc = bacc.Bacc("TRN2", debug=False)

    x = nc.dram_tensor("x", [BLOC, C, HW], F16, kind="ExternalInput")
    w0 = nc.dram_tensor("w0", [128, 1], F16, kind="ExternalInput")
    w1 = nc.dram_tensor("w1", [128, 1], F16, kind="ExternalInput")
    ones1 = nc.dram_tensor("ones1", [1, 128], F16, kind="ExternalInput")
    bias1 = nc.dram_tensor("bias1", [1, 1], F32, kind="ExternalInput")
    m2 = nc.dram_tensor("m2", [NBLK, 1, FD], F16, kind="ExternalInput")
    out = nc.dram_tensor("out", [BLOC, C, HW], F16, kind="ExternalOutput")

    # view [BLOC, C, HW] as [BLOC, p=128, h=2, n]: c = h*128 + p
    x_r = x.ap().rearrange("b (h p) n -> b p h n", h=2)
    out_r = out.ap().rearrange("b (h p) n -> b p h n", h=2)

    with tile.TileContext(nc) as tc:
        with (
            tc.tile_pool(name="consts", bufs=1) as consts,
            tc.tile_pool(name="xin", bufs=4) as xin_pool,
            tc.tile_pool(name="oout", bufs=2) as out_pool,
            tc.tile_pool(name="spool", bufs=2) as s_pool,
            tc.tile_pool(name="s2pool", bufs=2) as s2_pool,
            tc.tile_pool(name="cpool", bufs=2) as c_pool,
            tc.tile_pool(name="m2p", bufs=2) as m2_pool,
            tc.tile_pool(name="psA", bufs=4, space="PSUM") as psA,
            tc.tile_pool(name="psB", bufs=4, space="PSUM") as psB,
        ):
            w0_t = consts.tile([128, 1], F16)
            nc.sync.dma_start(out=w0_t[:], in_=w0.ap())
            w1_t = consts.tile([128, 1], F16)
            nc.sync.dma_start(out=w1_t[:], in_=w1.ap())
            ones1_t = consts.tile([1, 128], F16)
            nc.sync.dma_start(out=ones1_t[:], in_=ones1.ap())
            bias1_t = consts.tile([1, 1], F32)
            nc.sync.dma_start(out=bias1_t[:], in_=bias1.ap())

            for blk in range(NBLK):
                n0 = blk * FD
                m2_t = m2_pool.tile([1, FD], F16)
                nc.gpsimd.dma_start(out=m2_t[:], in_=m2.ap()[blk])
                for b in range(BLOC):
                    xt = xin_pool.tile([128, 2, FD], F16)
                    nc.sync.dma_start(out=xt[:], in_=x_r[b, :, :, n0:n0 + FD])
                    ot = out_pool.tile([128, 2, FD], F16)
                    st = s_pool.tile([1, FD], F16)
                    s2 = s2_pool.tile([1, FD], F16)
                    cmb = c_pool.tile([128, FD], F16)

                    for j in range(NSUB):
                        js = slice(j * SUB, (j + 1) * SUB)
                        ps_att = psA.tile([1, SUB], F32)
                        nc.tensor.matmul(
                            ps_att[:], w0_t[:], xt[:, 0, js],
                            start=True, stop=False,
                        )
                        nc.tensor.matmul(
                            ps_att[:], w1_t[:], xt[:, 1, js],
                            start=False, stop=True,
                        )
                        nc.scalar.activation(
                            out=st[:, js],
                            in_=ps_att[:],
                            func=mybir.ActivationFunctionType.Sigmoid,
                            bias=bias1_t[:],
                            scale=1.0,
                        )
                        nc.vector.tensor_mul(s2[:, js], st[:, js], m2_t[:, js])
                        ps_bc = psB.tile([128, SUB], F32)
                        nc.tensor.matmul(
                            ps_bc[:], ones1_t[:], s2[:, js],
                            start=True, stop=True,
                        )
                        # f32 PSUM -> fp16 SBUF cast; alternate engines so
                        # neither ACT nor DVE exceeds the per-block DMA window.
                        if j % 2 == 0:
                            nc.scalar.copy(cmb[:, js], ps_bc[:])
                        else:
                            nc.vector.tensor_copy(cmb[:, js], ps_bc[:])

                    for half in range(2):
                        hs = slice(half * HALF, (half + 1) * HALF)
                        nc.vector.tensor_mul(
                            ot[:, 0, hs], xt[:, 0, hs], cmb[:, hs])
                        nc.vector.tensor_mul(
                            ot[:, 1, hs], xt[:, 1, hs], cmb[:, hs])
                        nc.scalar.dma_start(
                            out=out_r[b, :, :, n0 + half * HALF:
                                      n0 + (half + 1) * HALF],
                            in_=ot[:, :, hs])

    nc.compile()
    return nc


def _host_consts(conv_w, conv_b):
    w = np.asarray(conv_w, dtype=np.float32).reshape(C).astype(np.float16)
    w0 = w[:128, None].copy()                              # [128, 1]
    w1 = w[128:, None].copy()                              # [128, 1]
    ones1 = np.ones((1, 128), dtype=np.float16)            # [1, 128]
    bias1 = np.full((1, 1), np.asarray(conv_b).reshape(-1)[0], dtype=np.float32)

    ys = np.arange(H)[:, None]
    xs = np.arange(W)[None, :]
    border = (ys == 0) | (ys == H - 1) | (xs == 0) | (xs == W - 1)
    m2vec = np.where(border, 2.0, 1.0).astype(np.float16).reshape(HW)
    m2 = m2vec.reshape(NBLK, 1, FD).copy()
    return dict(w0=w0, w1=w1, ones1=ones1, bias1=bias1, m2=m2)


def kernel(x, conv_w, conv_b):
    global LAST_RESULTS
    x = np.asarray(x)
    assert x.shape == (B, C, H, W), x.shape

    if "nc" not in _NC_CACHE:
        _NC_CACHE["nc"] = _build_nc()
    nc = _NC_CACHE["nc"]

    consts = _host_consts(conv_w, conv_b)
    x_flat = x.reshape(B, C, HW)

    in_maps = []
    for i in range(NCORES):
        xs16 = np.ascontiguousarray(
            x_flat[i * BLOC:(i + 1) * BLOC]).astype(np.float16)
        m = {"x": xs16}
        m.update(consts)
        in_maps.append(m)

    res = run_bass_kernel_spmd(nc, in_maps, list(range(NCORES)))
    LAST_RESULTS = res

    out = np.concatenate(
        [r["out"].reshape(BLOC, C, H, W) for r in res.results], axis=0
    ).astype(np.float32)
    return out
